# revision 6
# baseline (speedup 1.0000x reference)
"""Trainium2 Bass kernel for SegmentationAugmentation (3D affine grid_sample, trilinear, border).

Contract: kernel(input_g, label_g, transform) -> (aug_inp f32 [8,1,128,128,128],
                                                  aug_lab bool [8,1,128,128,128])

Math (transposes folded; all spatial dims 128):
  out[b,c,i,j,k] = trilinear sample of input_g[b,c,:,:,:] at
      p-axis: U(i,j) = clip(64*(a00*xn(i)+a01*xn(j)+a03)+63.5, 0, 127)
      q-axis: V(i,j) = clip(64*(a10*xn(i)+a11*xn(j)+a13)+63.5, 0, 127)
      r-axis: W(k)   = clip(64*(a22*xn(k)+a23)+63.5, 0, 127)
  with xn(t) = (2t+1)/128 - 1, theta = transform[:3]. Relies on the generator's
  z-rotation structure (theta[0:2,2]==0, theta[2,0:2]==0): U,V independent of k,
  W independent of (i,j). Host fallback handles arbitrary transforms.

Device pipeline (data parallel: core b handles batch b), fp16 end to end:
  Host packs volquad[p*128+q] = [V0[p,q,:], V1[p,q,:], V0[p1,q,:], V1[p1,q,:]]
  (fp16, p1 = min(p+1,127)), so ONE 2 KiB gather descriptor per output column
  (i,j) fetches rows (p0,q0),(p0,q0+1) = all 8 z-interp source rows for BOTH
  volumes and all 4 (p,q) corners. 16 dma_gather calls x 1024 descriptors total
  (the previous design needed 4x the descriptors; SWDGE desc-gen was the
  bottleneck at ~9 ns/descriptor).
  DVE per call: 4-corner mix (w00..w11 per-column broadcast weights; the
  (vol, p-corner) adjacency in the packed row makes (v,r) one contiguous
  256-elt dim, so every op is a 3-dim AP), then z-staircase out[k] =
  R[r0(k)]*(1-fw) + R[r1(k)]*fw over run-segmented +-1-stride slices.
  ACT engine streams fp16 outputs to DRAM; host upcasts aug_inp to f32.
Label bool: host thresholds fp16 label at 0.5; voxels within BAND=6e-3 of 0.5
(device arithmetic error is <=1.4e-3) are recomputed in the reference's exact
f32 arithmetic order.
"""
import numpy as np

N = 128
NROWS = N * N            # 16384 output columns (i,j), also volquad logical rows
NROWSQ = 16512           # padded volquad rows (>= 16384 + 2)
NCALL = 16               # gather calls per volume-pair
NIDX = 1024              # descriptors per call
SLOTS = NIDX // 128      # 8 descriptors per partition per call
ELEM = 1024              # fp16 elements per descriptor (2 volquad rows = 2 KiB)
ESTEP = 512              # volquad row stride in fp16 elements
BAND = np.float32(6e-3)  # label fixup band (>=4x max device deviation)

_CACHE = {}


def _host_tables(theta):
    """Transform tables in the reference's exact f32 arithmetic order."""
    f32 = np.float32
    t = np.arange(N, dtype=f32)
    xn = ((f32(2.0) * t + f32(1.0)) / f32(N) - f32(1.0)).astype(f32)
    th = theta.astype(f32)

    def fma32(a, b, c):
        return (np.float64(a) * np.float64(b) + np.float64(c)).astype(f32)

    ii = xn[:, None]
    jj = xn[None, :]
    U_g = fma32(jj, th[0, 1], (ii * th[0, 0]).astype(f32)) + th[0, 3]
    V_g = fma32(jj, th[1, 1], (ii * th[1, 0]).astype(f32)) + th[1, 3]
    W_g = (fma32(xn, th[2, 2], np.zeros_like(xn)) + th[2, 3]).astype(f32)

    def unnorm(c):
        return np.clip(((c.astype(f32) + f32(1.0)) * f32(N) - f32(1.0)) * f32(0.5),
                       f32(0.0), f32(N - 1))

    U, V, W = unnorm(U_g), unnorm(V_g), unnorm(W_g)
    p0 = np.floor(U).astype(np.int64)
    q0 = np.floor(V).astype(np.int64)
    r0 = np.floor(W).astype(np.int64)
    fu = (U - p0).astype(f32)
    fv = (V - q0).astype(f32)
    fw = (W - r0).astype(f32)
    r1 = np.minimum(r0 + 1, N - 1)

    idx = (p0 * 128 + q0).astype(np.int16)           # [i,j]
    w00 = ((1 - fu) * (1 - fv)).astype(f32)          # (p0,q0)
    w01 = ((1 - fu) * fv).astype(f32)                # (p0,q0+1)
    w10 = (fu * (1 - fv)).astype(f32)                # (p1,q0)
    w11 = (fu * fv).astype(f32)                      # (p1,q0+1)

    # z-runs: maximal k-segments where both r0,r1 step by a constant d in -1..1
    runs = []
    k = 0
    while k < N:
        step = 0
        if k + 1 < N:
            d = int(r0[k + 1] - r0[k])
            if d == int(r1[k + 1] - r1[k]) and abs(d) <= 1:
                step = d
        ln = 1
        while (k + ln < N
               and int(r0[k + ln] - r0[k]) == step * ln
               and int(r1[k + ln] - r1[k]) == step * ln):
            ln += 1
        runs.append((k, ln, int(r0[k]), int(r1[k]), step))
        k += ln

    return dict(idx=idx, w00=w00, w01=w01, w10=w10, w11=w11, fw=fw, runs=runs)


def _pack_idxs(idx_flat):
    """int16 dma_gather index layout: element i at [i%16, i//16], replicated to 128 partitions."""
    t = idx_flat.reshape(-1, 16).T.astype(np.int16)  # [16, n/16]
    return np.ascontiguousarray(np.tile(t, (8, 1)))  # [128, n/16]


def _pack_wbig(tables):
    """[16384, 1024] fp16 broadcast mix weights, row n (= i*128+j) =
    [w00 x256, w10 x256, w01 x256, w11 x256] matching the gathered element
    layout [q0:(v,p0),(v,p1) | q1:(v,p0),(v,p1)]. Materialized so the DVE mix
    is ONE packed fp16 multiply (2x mode needs every operand 2-byte with
    innermost stride +-1; an f32 stride-0 broadcast AP runs at 1x)."""
    w = np.stack([tables[nm].reshape(-1) for nm in ("w00", "w10", "w01", "w11")],
                 axis=1).astype(np.float16)          # [n, 4]
    return np.ascontiguousarray(
        np.broadcast_to(w[:, :, None], (NROWS, 4, 256)).reshape(NROWS, 1024))


def _build_volquad(v0, v1):
    """v0,v1: [128,128,128] f32. Returns [16512, 512] fp16 with row (p,q) =
    [v0[p,q,:], v1[p,q,:], v0[p1,q,:], v1[p1,q,:]], p1 = min(p+1, 127)."""
    p1 = np.minimum(np.arange(N) + 1, N - 1)
    vq = np.empty((N, N, 4, N), dtype=np.float16)
    vq[:, :, 0] = v0
    vq[:, :, 1] = v1
    vq[:, :, 2] = v0[p1]
    vq[:, :, 3] = v1[p1]
    out = np.zeros((NROWSQ, 4 * N), dtype=np.float16)
    out[:NROWS] = vq.reshape(NROWS, 4 * N)
    return out


def _build_program(tables, reps=1):
    """Raw-Bass program with explicit semaphores.

    Engine streams:
      sync   (SP HWDGE): const loads (idx, weights, fw tables)
      gpsimd (SWDGE):    one dma_gather per call (1024 x 2KiB descriptors)
      vector (DVE):      4-corner mix + z-staircase
      scalar (ACT HWDGE): fp16 output writes
    """
    import concourse.bass as bass
    from concourse import bacc, mybir

    runs = tables["runs"]
    f16 = mybir.dt.float16
    i16 = mybir.dt.int16
    AP = bass.AP

    nc = bacc.Bacc("TRN2", target_bir_lowering=False, debug=False, num_devices=8)

    volquad = nc.dram_tensor("volquad", [NROWSQ, 4 * N], f16, kind="ExternalInput")
    idx_dram = nc.dram_tensor("idx", [128, NROWS // 16], i16, kind="ExternalInput")
    wb_dram = nc.dram_tensor("wbig", [NROWS, 1024], f16, kind="ExternalInput")
    fw_dram = nc.dram_tensor("fwrep", [2, 128, 128], f16, kind="ExternalInput")
    out_dram = [nc.dram_tensor(f"out{v}", [NROWS, N], f16, kind="ExternalOutput")
                for v in range(2)]

    idx_t = nc.alloc_sbuf_tensor("idx_t", [128, NROWS // 16], i16)
    fw_t = [nc.alloc_sbuf_tensor(f"fw{c}", [128, 128], f16) for c in range(2)]
    NB = 3
    G = [nc.alloc_sbuf_tensor(f"G{p}", [128, SLOTS * ELEM], f16) for p in range(NB)]
    WB = [nc.alloc_sbuf_tensor(f"WB{p}", [128, SLOTS * ELEM], f16) for p in range(NB)]
    M = nc.alloc_sbuf_tensor("M", [128, SLOTS * ELEM], f16)
    A1 = nc.alloc_sbuf_tensor("A1", [128, SLOTS * 512], f16)
    R0 = nc.alloc_sbuf_tensor("R0", [128, SLOTS * 256], f16)
    t1 = nc.alloc_sbuf_tensor("t1", [128, SLOTS * 256], f16)
    acc = [nc.alloc_sbuf_tensor(f"acc{p}", [128, SLOTS * 256], f16) for p in range(2)]

    TOT = NCALL * reps
    nrows_ap = NROWSQ - 2 + 1

    from contextlib import ExitStack
    with ExitStack() as _sctx:
        block = _sctx.enter_context(nc.Block())
        s_wf = _sctx.enter_context(nc.semaphore("s_wf"))
        s_g = [_sctx.enter_context(nc.semaphore(f"s_g{p}")) for p in range(NB)]
        s_wb = [_sctx.enter_context(nc.semaphore(f"s_wb{p}")) for p in range(NB)]
        s_v = _sctx.enter_context(nc.semaphore("s_v"))
        s_mix = _sctx.enter_context(nc.semaphore("s_mix"))
        s_o = [_sctx.enter_context(nc.semaphore(f"s_o{p}")) for p in range(2)]

        @block.sync
        def _(sync):
            sync.dma_start(idx_t.ap(), idx_dram.ap()).then_inc(s_wf, 16)
            for c in range(2):
                sync.dma_start(fw_t[c].ap(),
                               AP(fw_dram, c * 128 * 128, [[128, 128], [1, 128]])
                               ).then_inc(s_wf, 16)
            for C in range(TOT):
                cl = C % NCALL
                if C >= NB:
                    sync.wait_ge(s_mix, C - NB + 1)
                sync.dma_start(
                    AP(WB[C % NB], 0, [[SLOTS * ELEM, 128], [ELEM, SLOTS], [1, ELEM]]),
                    AP(wb_dram, cl * NIDX * 1024,
                       [[1024, 128], [128 * 1024, SLOTS], [1, 1024]]),
                ).then_inc(s_wb[C % NB], 16)
            for p in range(2):
                sync.wait_ge(s_o[p], 32 * (TOT // 2))

        @block.gpsimd
        def _(gpsimd):
            nreg = gpsimd.to_reg(NIDX)
            gpsimd.wait_ge(s_wf, 48)
            sv = AP(volquad, 0, [[ESTEP, nrows_ap], [1, ELEM]])
            for C in range(TOT):
                cl = C % NCALL
                if C >= NB:
                    gpsimd.wait_ge(s_mix, C - NB + 1)
                gpsimd.dma_gather(
                    AP(G[C % NB], 0, [[SLOTS * ELEM, 128], [ELEM, SLOTS], [1, ELEM]]),
                    sv,
                    AP(idx_t, cl * (NIDX // 16), [[NROWS // 16, 128], [1, NIDX // 16]]),
                    NIDX, nreg, ELEM, elem_step=ESTEP,
                ).then_inc(s_g[C % NB], 16)

        @block.vector
        def _(vector):
            mult = mybir.AluOpType.mult
            VC = [0]

            def vsync(last_ins):
                # DVE pipeline does not interlock same-engine RAW hazards;
                # self-semaphore between dependent phases.
                last_ins.then_inc(s_v, 1)
                VC[0] += 1
                vector.wait_ge(s_v, VC[0])

            vector.wait_ge(s_wf, 48)
            flat = [[SLOTS * ELEM, 128], [1, SLOTS * ELEM]]
            for C in range(TOT):
                vector.wait_ge(s_g[C % NB], 16 * (C // NB + 1))
                vector.wait_ge(s_wb[C % NB], 16 * (C // NB + 1))
                if C >= 1:
                    # M/A1/R0/t1 WAR vs previous call (same-engine pipeline)
                    vector.wait_ge(s_mix, C)
                if C >= 2:
                    vector.wait_ge(s_o[C % 2], 32 * (C // 2))
                A = acc[C % 2]
                # mix: one packed fp16 multiply (2x mode), then pair-sum tree
                vsync(vector.tensor_tensor(AP(M, 0, flat), AP(G[C % NB], 0, flat),
                                           AP(WB[C % NB], 0, flat), mult))
                vsync(vector.tensor_add(
                    AP(A1, 0, [[SLOTS * 512, 128], [512, SLOTS], [1, 512]]),
                    AP(M, 0, [[SLOTS * ELEM, 128], [ELEM, SLOTS], [1, 512]]),
                    AP(M, 512, [[SLOTS * ELEM, 128], [ELEM, SLOTS], [1, 512]])))
                vsync(vector.tensor_add(
                    AP(R0, 0, [[SLOTS * 256, 128], [256, SLOTS], [1, 256]]),
                    AP(A1, 0, [[SLOTS * 512, 128], [512, SLOTS], [1, 256]]),
                    AP(A1, 256, [[SLOTS * 512, 128], [512, SLOTS], [1, 256]])))
                # z-staircase: view R0/acc as [128, 16 (slot*v), 128 (r|k)]
                last = None
                for (ks, ln, r0s, r1s, st) in runs:
                    adst = AP(A, ks, [[SLOTS * 256, 128], [128, 2 * SLOTS], [1, ln]])
                    tdst = AP(t1, ks, [[SLOTS * 256, 128], [128, 2 * SLOTS], [1, ln]])
                    v0 = AP(R0, r0s, [[SLOTS * 256, 128], [128, 2 * SLOTS], [st, ln]])
                    v1 = AP(R0, r1s, [[SLOTS * 256, 128], [128, 2 * SLOTS], [st, ln]])
                    f0 = AP(fw_t[0], ks, [[128, 128], [0, 2 * SLOTS], [1, ln]])
                    f1 = AP(fw_t[1], ks, [[128, 128], [0, 2 * SLOTS], [1, ln]])
                    vector.tensor_tensor(adst, v0, f0, mult)
                    last = vector.tensor_tensor(tdst, v1, f1, mult)
                vsync(last)
                for (ks, ln, r0s, r1s, st) in runs:
                    adst = AP(A, ks, [[SLOTS * 256, 128], [128, 2 * SLOTS], [1, ln]])
                    tsrc = AP(t1, ks, [[SLOTS * 256, 128], [128, 2 * SLOTS], [1, ln]])
                    last = vector.tensor_add(adst, adst, tsrc)
                last.then_inc(s_mix, 1)

        @block.scalar
        def _(scalar):
            for C in range(TOT):
                cl = C % NCALL
                scalar.wait_ge(s_mix, C + 1)
                for v in range(2):
                    scalar.dma_start(
                        AP(out_dram[v], cl * NIDX * N, [[N, 128], [128 * N, SLOTS], [1, N]]),
                        AP(acc[C % 2], v * 128, [[SLOTS * 256, 128], [256, SLOTS], [1, N]]),
                    ).then_inc(s_o[C % 2], 16)

    nc.compile()
    return nc


def _exact_label_fixup(label_g, theta, lab_f, out_bool):
    """Recompute voxels of |lab_f - 0.5| < BAND in the reference's exact f32
    arithmetic order."""
    cand = np.abs(lab_f - np.float32(0.5)) < BAND
    if not cand.any():
        return out_bool
    bb, ii, jj, kk = np.nonzero(cand.reshape(-1, N, N, N))
    v = _exact_reference_values(label_g, theta, bb, ii, jj, kk)
    out_bool.reshape(-1, N, N, N)[bb, ii, jj, kk] = v > np.float32(0.5)
    return out_bool


def _exact_reference_values(vol_g, theta, bb, ii, jj, kk):
    """Reference-order f32 trilinear values at selected voxels.

    Replicates: grid einsum (x*t0 + y*t1 + z*t2, left-assoc f32) + t3; unnorm;
    8-corner accumulation in (z,y,x) order with w=(wz*wy)*wx, out += v*w.
    """
    f32 = np.float32
    t = np.arange(N, dtype=f32)
    xn = ((f32(2.0) * t + f32(1.0)) / f32(N) - f32(1.0)).astype(f32)
    th = theta.astype(f32)

    x = xn[ii]; y = xn[jj]; z = xn[kk]

    def fma32(a, b, c):
        return (np.float64(a) * np.float64(b) + c.astype(np.float64)).astype(f32)

    def comp(r):
        a = fma32(y, th[r, 1], (x * th[r, 0]).astype(f32))
        a = fma32(z, th[r, 2], a)
        return (a + th[r, 3]).astype(f32)
    gx, gy, gz = comp(0), comp(1), comp(2)

    def unnorm(c):
        return np.clip(((c + f32(1.0)) * f32(N) - f32(1.0)) * f32(0.5), f32(0.0), f32(N - 1))
    ux, uy, uz = unnorm(gx), unnorm(gy), unnorm(gz)
    x0 = np.floor(ux); y0 = np.floor(uy); z0 = np.floor(uz)
    fx = (ux - x0).astype(f32); fy = (uy - y0).astype(f32); fz = (uz - z0).astype(f32)
    x0i = x0.astype(np.int64); y0i = y0.astype(np.int64); z0i = z0.astype(np.int64)
    x1i = np.minimum(x0i + 1, N - 1); y1i = np.minimum(y0i + 1, N - 1); z1i = np.minimum(z0i + 1, N - 1)

    vol = vol_g.reshape(-1, N, N, N)
    out = np.zeros(bb.shape, f32)
    one = f32(1.0)
    for zi, wz in ((z0i, (one - fz).astype(f32)), (z1i, fz)):
        for yi, wy in ((y0i, (one - fy).astype(f32)), (y1i, fy)):
            for xi, wx in ((x0i, (one - fx).astype(f32)), (x1i, fx)):
                vals = vol[bb, xi, yi, zi]
                w = ((wz * wy).astype(f32) * wx).astype(f32)
                out = (out + (vals * w).astype(f32)).astype(f32)
    return out


def _host_fallback(input_g, label_g, transform):
    """Arbitrary-transform fallback: full reference computation on host."""
    bb, ii, jj, kk = np.meshgrid(np.arange(8), np.arange(N), np.arange(N), np.arange(N), indexing="ij")
    bb, ii, jj, kk = (a.reshape(-1) for a in (bb, ii, jj, kk))
    theta = transform[:3].astype(np.float32)
    aug_inp = _exact_reference_values(input_g, theta, bb, ii, jj, kk).reshape(8, 1, N, N, N)
    lab = _exact_reference_values(label_g, theta, bb, ii, jj, kk).reshape(8, 1, N, N, N)
    return aug_inp.astype(np.float32), lab > np.float32(0.5)


def _make_in_maps(input_g, label_g, tables):
    common = {
        "idx": _pack_idxs(tables["idx"].reshape(-1)),
        "wbig": _pack_wbig(tables),
        "fwrep": np.stack([np.tile(1.0 - tables["fw"], (128, 1)),
                           np.tile(tables["fw"], (128, 1))]).astype(np.float16),
    }
    return [dict(common, volquad=_build_volquad(input_g[b, 0], label_g[b, 0]))
            for b in range(8)]


def kernel(input_g, label_g, transform):
    input_g = np.ascontiguousarray(input_g, dtype=np.float32)
    label_g = np.ascontiguousarray(label_g, dtype=np.float32)
    transform = np.asarray(transform, dtype=np.float32)
    theta = transform[:3]

    structured = (abs(float(theta[0, 2])) < 1e-12 and abs(float(theta[1, 2])) < 1e-12
                  and abs(float(theta[2, 0])) < 1e-12 and abs(float(theta[2, 1])) < 1e-12)
    if not structured:
        return _host_fallback(input_g, label_g, transform)

    from concourse.bass_utils import run_bass_kernel_spmd

    tables = _host_tables(theta)
    key = transform.tobytes()
    if key not in _CACHE:
        _CACHE[key] = _build_program(tables)
    nc = _CACHE[key]

    in_maps = _make_in_maps(input_g, label_g, tables)
    res = run_bass_kernel_spmd(nc, in_maps, core_ids=list(range(8)))

    aug_inp = np.empty((8, 1, N, N, N), np.float32)
    lab_f = np.empty((8, 1, N, N, N), np.float32)
    for b in range(8):
        aug_inp[b, 0] = res.results[b]["out0"].astype(np.float32).reshape(N, N, N)
        lab_f[b, 0] = res.results[b]["out1"].astype(np.float32).reshape(N, N, N)

    out_bool = lab_f > np.float32(0.5)
    out_bool = _exact_label_fixup(label_g, theta, lab_f, out_bool)
    return aug_inp, out_bool


# revision 9
# speedup vs baseline: 1.1485x; 1.1485x over previous
"""Trainium2 Bass kernel for SegmentationAugmentation (3D affine grid_sample, trilinear, border).

Contract: kernel(input_g, label_g, transform) -> (aug_inp f32 [8,1,128,128,128],
                                                  aug_lab bool [8,1,128,128,128])

Math (transposes folded; all spatial dims 128):
  out[b,c,i,j,k] = trilinear sample of input_g[b,c,:,:,:] at
      p-axis: U(i,j) = clip(64*(a00*xn(i)+a01*xn(j)+a03)+63.5, 0, 127)
      q-axis: V(i,j) = clip(64*(a10*xn(i)+a11*xn(j)+a13)+63.5, 0, 127)
      r-axis: W(k)   = clip(64*(a22*xn(k)+a23)+63.5, 0, 127)
  with xn(t) = (2t+1)/128 - 1, theta = transform[:3]. Relies on the generator's
  z-rotation structure (theta[0:2,2]==0, theta[2,0:2]==0): U,V independent of k,
  W independent of (i,j). Host fallback handles arbitrary transforms.

Device pipeline (data parallel: core b handles batch b), fp16 end to end:
  Host packs volquad[p*128+q] = [V0[p,q,:], V1[p,q,:], V0[p1,q,:], V1[p1,q,:]]
  (fp16, p1 = min(p+1,127)), so ONE 2 KiB gather descriptor per output column
  (i,j) fetches rows (p0,q0),(p0,q0+1) = all 8 z-interp source rows for BOTH
  volumes and all 4 (p,q) corners. 16 dma_gather calls x 1024 descriptors total
  (the previous design needed 4x the descriptors; SWDGE desc-gen was the
  bottleneck at ~9 ns/descriptor).
  DVE per call: 4-corner mix (w00..w11 per-column broadcast weights; the
  (vol, p-corner) adjacency in the packed row makes (v,r) one contiguous
  256-elt dim, so every op is a 3-dim AP), then z-staircase out[k] =
  R[r0(k)]*(1-fw) + R[r1(k)]*fw over run-segmented +-1-stride slices.
  ACT engine streams fp16 outputs to DRAM; host upcasts aug_inp to f32.
Label bool: host thresholds fp16 label at 0.5; voxels within BAND=6e-3 of 0.5
(device arithmetic error is <=1.4e-3) are recomputed in the reference's exact
f32 arithmetic order.
"""
import numpy as np

N = 128
NROWS = N * N            # 16384 output columns (i,j), also volquad logical rows
NROWSQ = 16512           # padded volquad rows (>= 16384 + 2)
NCALL = 16               # gather calls per volume-pair
NIDX = 1024              # descriptors per call
SLOTS = NIDX // 128      # 8 descriptors per partition per call
ELEM = 1024              # fp16 elements per descriptor (2 volquad rows = 2 KiB)
ESTEP = 512              # volquad row stride in fp16 elements
BAND = np.float32(6e-3)  # label fixup band (>=4x max device deviation)

_CACHE = {}


def _host_tables(theta):
    """Transform tables in the reference's exact f32 arithmetic order."""
    f32 = np.float32
    t = np.arange(N, dtype=f32)
    xn = ((f32(2.0) * t + f32(1.0)) / f32(N) - f32(1.0)).astype(f32)
    th = theta.astype(f32)

    def fma32(a, b, c):
        return (np.float64(a) * np.float64(b) + np.float64(c)).astype(f32)

    ii = xn[:, None]
    jj = xn[None, :]
    U_g = fma32(jj, th[0, 1], (ii * th[0, 0]).astype(f32)) + th[0, 3]
    V_g = fma32(jj, th[1, 1], (ii * th[1, 0]).astype(f32)) + th[1, 3]
    W_g = (fma32(xn, th[2, 2], np.zeros_like(xn)) + th[2, 3]).astype(f32)

    def unnorm(c):
        return np.clip(((c.astype(f32) + f32(1.0)) * f32(N) - f32(1.0)) * f32(0.5),
                       f32(0.0), f32(N - 1))

    U, V, W = unnorm(U_g), unnorm(V_g), unnorm(W_g)
    p0 = np.floor(U).astype(np.int64)
    q0 = np.floor(V).astype(np.int64)
    r0 = np.floor(W).astype(np.int64)
    fu = (U - p0).astype(f32)
    fv = (V - q0).astype(f32)
    fw = (W - r0).astype(f32)
    r1 = np.minimum(r0 + 1, N - 1)

    idx = (p0 * 128 + q0).astype(np.int16)           # [i,j]
    w00 = ((1 - fu) * (1 - fv)).astype(f32)          # (p0,q0)
    w01 = ((1 - fu) * fv).astype(f32)                # (p0,q0+1)
    w10 = (fu * (1 - fv)).astype(f32)                # (p1,q0)
    w11 = (fu * fv).astype(f32)                      # (p1,q0+1)

    # z-runs: maximal k-segments where both r0,r1 step by a constant d in -1..1
    runs = []
    k = 0
    while k < N:
        step = 0
        if k + 1 < N:
            d = int(r0[k + 1] - r0[k])
            if d == int(r1[k + 1] - r1[k]) and abs(d) <= 1:
                step = d
        ln = 1
        while (k + ln < N
               and int(r0[k + ln] - r0[k]) == step * ln
               and int(r1[k + ln] - r1[k]) == step * ln):
            ln += 1
        runs.append((k, ln, int(r0[k]), int(r1[k]), step))
        k += ln

    return dict(idx=idx, w00=w00, w01=w01, w10=w10, w11=w11, fw=fw, runs=runs)


def _pack_idxs(idx_flat):
    """int16 dma_gather index layout: element i at [i%16, i//16], replicated to 128 partitions."""
    t = idx_flat.reshape(-1, 16).T.astype(np.int16)  # [16, n/16]
    return np.ascontiguousarray(np.tile(t, (8, 1)))  # [128, n/16]


def _pack_wsm(tables):
    """[16384, 512] fp16 mix weights, row n (= i*128+j) =
    [w00 x128, w10 x128, w01 x128, w11 x128] (order matches the gathered
    element's (q,p)-corner slices at strides of 256). Materialized over r so
    the DVE mix multiply runs in fp16 2x mode (every operand needs 2-byte
    dtype with a stride-1 innermost dim; the v-broadcast uses a stride-0
    MIDDLE dim, which 2x mode permits)."""
    w = np.stack([tables[nm].reshape(-1) for nm in ("w00", "w10", "w01", "w11")],
                 axis=1).astype(np.float16)          # [n, 4]
    return np.ascontiguousarray(
        np.broadcast_to(w[:, :, None], (NROWS, 4, 128)).reshape(NROWS, 512))


def _build_volquad(v0, v1):
    """v0,v1: [128,128,128] f32. Returns [16512, 512] fp16 with row (p,q) =
    [v0[p,q,:], v1[p,q,:], v0[p1,q,:], v1[p1,q,:]], p1 = min(p+1, 127)."""
    p1 = np.minimum(np.arange(N) + 1, N - 1)
    vq = np.empty((N, N, 4, N), dtype=np.float16)
    vq[:, :, 0] = v0
    vq[:, :, 1] = v1
    vq[:, :, 2] = v0[p1]
    vq[:, :, 3] = v1[p1]
    out = np.zeros((NROWSQ, 4 * N), dtype=np.float16)
    out[:NROWS] = vq.reshape(NROWS, 4 * N)
    return out


def _build_program(tables, reps=1):
    """Raw-Bass program with explicit semaphores.

    Engine streams:
      sync   (SP HWDGE): const loads (idx, weights, fw tables)
      gpsimd (SWDGE):    one dma_gather per call (1024 x 2KiB descriptors)
      vector (DVE):      4-corner mix + z-staircase
      scalar (ACT HWDGE): fp16 output writes
    """
    import concourse.bass as bass
    from concourse import bacc, mybir

    runs = tables["runs"]
    f16 = mybir.dt.float16
    i16 = mybir.dt.int16
    AP = bass.AP

    nc = bacc.Bacc("TRN2", target_bir_lowering=False, debug=False, num_devices=8)

    volquad = nc.dram_tensor("volquad", [NROWSQ, 4 * N], f16, kind="ExternalInput")
    idx_dram = nc.dram_tensor("idx", [128, NROWS // 16], i16, kind="ExternalInput")
    wb_dram = nc.dram_tensor("wsm", [NROWS, 512], f16, kind="ExternalInput")
    fw_dram = nc.dram_tensor("fwrep", [2, 128, 128], f16, kind="ExternalInput")
    out_dram = [nc.dram_tensor(f"out{v}", [NROWS, N], f16, kind="ExternalOutput")
                for v in range(2)]

    idx_t = nc.alloc_sbuf_tensor("idx_t", [128, NROWS // 16], i16)
    fw_t = [nc.alloc_sbuf_tensor(f"fw{c}", [128, 128], f16) for c in range(2)]
    NB = 3
    G = [nc.alloc_sbuf_tensor(f"G{p}", [128, SLOTS * ELEM], f16) for p in range(NB)]
    WB = [nc.alloc_sbuf_tensor(f"WB{p}", [128, SLOTS * 512], f16) for p in range(NB)]
    M = nc.alloc_sbuf_tensor("M", [128, SLOTS * ELEM], f16)
    A1 = nc.alloc_sbuf_tensor("A1", [128, SLOTS * 512], f16)
    R0 = nc.alloc_sbuf_tensor("R0", [128, SLOTS * 256], f16)
    t1 = nc.alloc_sbuf_tensor("t1", [128, SLOTS * 256], f16)
    acc = [nc.alloc_sbuf_tensor(f"acc{p}", [128, SLOTS * 256], f16) for p in range(2)]

    TOT = NCALL * reps
    nrows_ap = NROWSQ - 2 + 1

    from contextlib import ExitStack
    with ExitStack() as _sctx:
        block = _sctx.enter_context(nc.Block())
        s_wf = _sctx.enter_context(nc.semaphore("s_wf"))
        s_g = [_sctx.enter_context(nc.semaphore(f"s_g{p}")) for p in range(NB)]
        s_wb = [_sctx.enter_context(nc.semaphore(f"s_wb{p}")) for p in range(NB)]
        s_v = _sctx.enter_context(nc.semaphore("s_v"))
        s_mix = _sctx.enter_context(nc.semaphore("s_mix"))
        s_o = [_sctx.enter_context(nc.semaphore(f"s_o{p}")) for p in range(2)]

        @block.sync
        def _(sync):
            sync.dma_start(idx_t.ap(), idx_dram.ap()).then_inc(s_wf, 16)
            for c in range(2):
                sync.dma_start(fw_t[c].ap(),
                               AP(fw_dram, c * 128 * 128, [[128, 128], [1, 128]])
                               ).then_inc(s_wf, 16)
            for C in range(TOT):
                cl = C % NCALL
                if C >= NB:
                    sync.wait_ge(s_mix, C - NB + 1)
                sync.dma_start(
                    AP(WB[C % NB], 0, [[SLOTS * 512, 128], [512, SLOTS], [1, 512]]),
                    AP(wb_dram, cl * NIDX * 512,
                       [[512, 128], [128 * 512, SLOTS], [1, 512]]),
                ).then_inc(s_wb[C % NB], 16)
            for p in range(2):
                sync.wait_ge(s_o[p], 32 * (TOT // 2))

        @block.gpsimd
        def _(gpsimd):
            nreg = gpsimd.to_reg(NIDX)
            gpsimd.wait_ge(s_wf, 48)
            sv = AP(volquad, 0, [[ESTEP, nrows_ap], [1, ELEM]])
            for C in range(TOT):
                cl = C % NCALL
                if C >= NB:
                    gpsimd.wait_ge(s_mix, C - NB + 1)
                gpsimd.dma_gather(
                    AP(G[C % NB], 0, [[SLOTS * ELEM, 128], [ELEM, SLOTS], [1, ELEM]]),
                    sv,
                    AP(idx_t, cl * (NIDX // 16), [[NROWS // 16, 128], [1, NIDX // 16]]),
                    NIDX, nreg, ELEM, elem_step=ESTEP,
                ).then_inc(s_g[C % NB], 16)

        @block.vector
        def _(vector):
            mult = mybir.AluOpType.mult
            VC = [0]

            def vsync(last_ins):
                # DVE pipeline does not interlock same-engine RAW hazards;
                # self-semaphore between dependent phases.
                last_ins.then_inc(s_v, 1)
                VC[0] += 1
                vector.wait_ge(s_v, VC[0])

            vector.wait_ge(s_wf, 48)
            mdims = [[SLOTS * ELEM, 128], [256, 4 * SLOTS], [128, 2], [1, 128]]
            wdims = [[SLOTS * 512, 128], [128, 4 * SLOTS], [0, 2], [1, 128]]
            for C in range(TOT):
                vector.wait_ge(s_g[C % NB], 16 * (C // NB + 1))
                vector.wait_ge(s_wb[C % NB], 16 * (C // NB + 1))
                if C >= 1:
                    # M/A1/R0/t1 WAR vs previous call (same-engine pipeline)
                    vector.wait_ge(s_mix, C)
                if C >= 2:
                    vector.wait_ge(s_o[C % 2], 32 * (C // 2))
                A = acc[C % 2]
                # mix: one packed fp16 multiply (2x mode), then pair-sum tree
                vsync(vector.tensor_tensor(AP(M, 0, mdims), AP(G[C % NB], 0, mdims),
                                           AP(WB[C % NB], 0, wdims), mult))
                vsync(vector.tensor_add(
                    AP(A1, 0, [[SLOTS * 512, 128], [512, SLOTS], [1, 512]]),
                    AP(M, 0, [[SLOTS * ELEM, 128], [ELEM, SLOTS], [1, 512]]),
                    AP(M, 512, [[SLOTS * ELEM, 128], [ELEM, SLOTS], [1, 512]])))
                vsync(vector.tensor_add(
                    AP(R0, 0, [[SLOTS * 256, 128], [256, SLOTS], [1, 256]]),
                    AP(A1, 0, [[SLOTS * 512, 128], [512, SLOTS], [1, 256]]),
                    AP(A1, 256, [[SLOTS * 512, 128], [512, SLOTS], [1, 256]])))
                # z-staircase: view R0/acc as [128, 16 (slot*v), 128 (r|k)]
                last = None
                for (ks, ln, r0s, r1s, st) in runs:
                    adst = AP(A, ks, [[SLOTS * 256, 128], [128, 2 * SLOTS], [1, ln]])
                    tdst = AP(t1, ks, [[SLOTS * 256, 128], [128, 2 * SLOTS], [1, ln]])
                    v0 = AP(R0, r0s, [[SLOTS * 256, 128], [128, 2 * SLOTS], [st, ln]])
                    v1 = AP(R0, r1s, [[SLOTS * 256, 128], [128, 2 * SLOTS], [st, ln]])
                    f0 = AP(fw_t[0], ks, [[128, 128], [0, 2 * SLOTS], [1, ln]])
                    f1 = AP(fw_t[1], ks, [[128, 128], [0, 2 * SLOTS], [1, ln]])
                    vector.tensor_tensor(adst, v0, f0, mult)
                    last = vector.tensor_tensor(tdst, v1, f1, mult)
                vsync(last)
                for (ks, ln, r0s, r1s, st) in runs:
                    adst = AP(A, ks, [[SLOTS * 256, 128], [128, 2 * SLOTS], [1, ln]])
                    tsrc = AP(t1, ks, [[SLOTS * 256, 128], [128, 2 * SLOTS], [1, ln]])
                    last = vector.tensor_add(adst, adst, tsrc)
                last.then_inc(s_mix, 1)

        @block.scalar
        def _(scalar):
            for C in range(TOT):
                cl = C % NCALL
                scalar.wait_ge(s_mix, C + 1)
                for v in range(2):
                    scalar.dma_start(
                        AP(out_dram[v], cl * NIDX * N, [[N, 128], [128 * N, SLOTS], [1, N]]),
                        AP(acc[C % 2], v * 128, [[SLOTS * 256, 128], [256, SLOTS], [1, N]]),
                    ).then_inc(s_o[C % 2], 16)

    nc.compile()
    return nc


def _exact_label_fixup(label_g, theta, lab_f, out_bool):
    """Recompute voxels of |lab_f - 0.5| < BAND in the reference's exact f32
    arithmetic order."""
    cand = np.abs(lab_f - np.float32(0.5)) < BAND
    if not cand.any():
        return out_bool
    bb, ii, jj, kk = np.nonzero(cand.reshape(-1, N, N, N))
    v = _exact_reference_values(label_g, theta, bb, ii, jj, kk)
    out_bool.reshape(-1, N, N, N)[bb, ii, jj, kk] = v > np.float32(0.5)
    return out_bool


def _exact_reference_values(vol_g, theta, bb, ii, jj, kk):
    """Reference-order f32 trilinear values at selected voxels.

    Replicates: grid einsum (x*t0 + y*t1 + z*t2, left-assoc f32) + t3; unnorm;
    8-corner accumulation in (z,y,x) order with w=(wz*wy)*wx, out += v*w.
    """
    f32 = np.float32
    t = np.arange(N, dtype=f32)
    xn = ((f32(2.0) * t + f32(1.0)) / f32(N) - f32(1.0)).astype(f32)
    th = theta.astype(f32)

    x = xn[ii]; y = xn[jj]; z = xn[kk]

    def fma32(a, b, c):
        return (np.float64(a) * np.float64(b) + c.astype(np.float64)).astype(f32)

    def comp(r):
        a = fma32(y, th[r, 1], (x * th[r, 0]).astype(f32))
        a = fma32(z, th[r, 2], a)
        return (a + th[r, 3]).astype(f32)
    gx, gy, gz = comp(0), comp(1), comp(2)

    def unnorm(c):
        return np.clip(((c + f32(1.0)) * f32(N) - f32(1.0)) * f32(0.5), f32(0.0), f32(N - 1))
    ux, uy, uz = unnorm(gx), unnorm(gy), unnorm(gz)
    x0 = np.floor(ux); y0 = np.floor(uy); z0 = np.floor(uz)
    fx = (ux - x0).astype(f32); fy = (uy - y0).astype(f32); fz = (uz - z0).astype(f32)
    x0i = x0.astype(np.int64); y0i = y0.astype(np.int64); z0i = z0.astype(np.int64)
    x1i = np.minimum(x0i + 1, N - 1); y1i = np.minimum(y0i + 1, N - 1); z1i = np.minimum(z0i + 1, N - 1)

    vol = vol_g.reshape(-1, N, N, N)
    out = np.zeros(bb.shape, f32)
    one = f32(1.0)
    for zi, wz in ((z0i, (one - fz).astype(f32)), (z1i, fz)):
        for yi, wy in ((y0i, (one - fy).astype(f32)), (y1i, fy)):
            for xi, wx in ((x0i, (one - fx).astype(f32)), (x1i, fx)):
                vals = vol[bb, xi, yi, zi]
                w = ((wz * wy).astype(f32) * wx).astype(f32)
                out = (out + (vals * w).astype(f32)).astype(f32)
    return out


def _host_fallback(input_g, label_g, transform):
    """Arbitrary-transform fallback: full reference computation on host."""
    bb, ii, jj, kk = np.meshgrid(np.arange(8), np.arange(N), np.arange(N), np.arange(N), indexing="ij")
    bb, ii, jj, kk = (a.reshape(-1) for a in (bb, ii, jj, kk))
    theta = transform[:3].astype(np.float32)
    aug_inp = _exact_reference_values(input_g, theta, bb, ii, jj, kk).reshape(8, 1, N, N, N)
    lab = _exact_reference_values(label_g, theta, bb, ii, jj, kk).reshape(8, 1, N, N, N)
    return aug_inp.astype(np.float32), lab > np.float32(0.5)


def _make_in_maps(input_g, label_g, tables):
    common = {
        "idx": _pack_idxs(tables["idx"].reshape(-1)),
        "wsm": _pack_wsm(tables),
        "fwrep": np.stack([np.tile(1.0 - tables["fw"], (128, 1)),
                           np.tile(tables["fw"], (128, 1))]).astype(np.float16),
    }
    return [dict(common, volquad=_build_volquad(input_g[b, 0], label_g[b, 0]))
            for b in range(8)]


def kernel(input_g, label_g, transform):
    input_g = np.ascontiguousarray(input_g, dtype=np.float32)
    label_g = np.ascontiguousarray(label_g, dtype=np.float32)
    transform = np.asarray(transform, dtype=np.float32)
    theta = transform[:3]

    structured = (abs(float(theta[0, 2])) < 1e-12 and abs(float(theta[1, 2])) < 1e-12
                  and abs(float(theta[2, 0])) < 1e-12 and abs(float(theta[2, 1])) < 1e-12)
    if not structured:
        return _host_fallback(input_g, label_g, transform)

    from concourse.bass_utils import run_bass_kernel_spmd

    tables = _host_tables(theta)
    key = transform.tobytes()
    if key not in _CACHE:
        _CACHE[key] = _build_program(tables)
    nc = _CACHE[key]

    in_maps = _make_in_maps(input_g, label_g, tables)
    res = run_bass_kernel_spmd(nc, in_maps, core_ids=list(range(8)))

    aug_inp = np.empty((8, 1, N, N, N), np.float32)
    lab_f = np.empty((8, 1, N, N, N), np.float32)
    for b in range(8):
        aug_inp[b, 0] = res.results[b]["out0"].astype(np.float32).reshape(N, N, N)
        lab_f[b, 0] = res.results[b]["out1"].astype(np.float32).reshape(N, N, N)

    out_bool = lab_f > np.float32(0.5)
    out_bool = _exact_label_fixup(label_g, theta, lab_f, out_bool)
    return aug_inp, out_bool


# revision 18
# speedup vs baseline: 1.1920x; 1.0379x over previous
"""Trainium2 Bass kernel for SegmentationAugmentation (3D affine grid_sample, trilinear, border).

Contract: kernel(input_g, label_g, transform) -> (aug_inp f32 [8,1,128,128,128],
                                                  aug_lab bool [8,1,128,128,128])

Math (transposes folded; all spatial dims 128):
  out[b,c,i,j,k] = trilinear sample of input_g[b,c,:,:,:] at
      p-axis: U(i,j) = clip(64*(a00*xn(i)+a01*xn(j)+a03)+63.5, 0, 127)
      q-axis: V(i,j) = clip(64*(a10*xn(i)+a11*xn(j)+a13)+63.5, 0, 127)
      r-axis: W(k)   = clip(64*(a22*xn(k)+a23)+63.5, 0, 127)
  with xn(t) = (2t+1)/128 - 1, theta = transform[:3]. Relies on the generator's
  z-rotation structure (theta[0:2,2]==0, theta[2,0:2]==0): U,V independent of k,
  W independent of (i,j). Host fallback handles arbitrary transforms.

Device pipeline (data parallel: core b handles batch b), fp16 end to end:
  Host packs volquad[p*128+q] = [V0[p,q,:], V1[p,q,:], V0[p1,q,:], V1[p1,q,:]]
  (fp16, p1 = min(p+1,127)), so ONE 2 KiB gather descriptor per output column
  (i,j) fetches rows (p0,q0),(p0,q0+1) = all 8 z-interp source rows for BOTH
  volumes and all 4 (p,q) corners. 16 dma_gather calls x 1024 descriptors total
  (the previous design needed 4x the descriptors; SWDGE desc-gen was the
  bottleneck at ~9 ns/descriptor).
  DVE per call: 4-corner mix (w00..w11 per-column broadcast weights; the
  (vol, p-corner) adjacency in the packed row makes (v,r) one contiguous
  256-elt dim, so every op is a 3-dim AP), then z-staircase out[k] =
  R[r0(k)]*(1-fw) + R[r1(k)]*fw over run-segmented +-1-stride slices.
  ACT engine streams fp16 outputs to DRAM; host upcasts aug_inp to f32.
Label bool: host thresholds fp16 label at 0.5; voxels within BAND=6e-3 of 0.5
(device arithmetic error is <=1.4e-3) are recomputed in the reference's exact
f32 arithmetic order.
"""
import numpy as np

N = 128
NROWS = N * N            # 16384 output columns (i,j), also volquad logical rows
NROWSQ = 16512           # padded volquad rows (>= 16384 + 2)
NCALL = 16               # gather calls per volume-pair
NIDX = 1024              # descriptors per call
SLOTS = NIDX // 128      # 8 descriptors per partition per call
ELEM = 1024              # fp16 elements per descriptor (2 volquad rows = 2 KiB)
ESTEP = 512              # volquad row stride in fp16 elements
BAND = np.float32(6e-3)  # label fixup band (>=4x max device deviation)

_CACHE = {}


def _host_tables(theta):
    """Transform tables in the reference's exact f32 arithmetic order."""
    f32 = np.float32
    t = np.arange(N, dtype=f32)
    xn = ((f32(2.0) * t + f32(1.0)) / f32(N) - f32(1.0)).astype(f32)
    th = theta.astype(f32)

    def fma32(a, b, c):
        return (np.float64(a) * np.float64(b) + np.float64(c)).astype(f32)

    ii = xn[:, None]
    jj = xn[None, :]
    U_g = fma32(jj, th[0, 1], (ii * th[0, 0]).astype(f32)) + th[0, 3]
    V_g = fma32(jj, th[1, 1], (ii * th[1, 0]).astype(f32)) + th[1, 3]
    W_g = (fma32(xn, th[2, 2], np.zeros_like(xn)) + th[2, 3]).astype(f32)

    def unnorm(c):
        return np.clip(((c.astype(f32) + f32(1.0)) * f32(N) - f32(1.0)) * f32(0.5),
                       f32(0.0), f32(N - 1))

    U, V, W = unnorm(U_g), unnorm(V_g), unnorm(W_g)
    p0 = np.floor(U).astype(np.int64)
    q0 = np.floor(V).astype(np.int64)
    r0 = np.floor(W).astype(np.int64)
    fu = (U - p0).astype(f32)
    fv = (V - q0).astype(f32)
    fw = (W - r0).astype(f32)
    r1 = np.minimum(r0 + 1, N - 1)

    idx = (p0 * 128 + q0).astype(np.int16)           # [i,j]
    w00 = ((1 - fu) * (1 - fv)).astype(f32)          # (p0,q0)
    w01 = ((1 - fu) * fv).astype(f32)                # (p0,q0+1)
    w10 = (fu * (1 - fv)).astype(f32)                # (p1,q0)
    w11 = (fu * fv).astype(f32)                      # (p1,q0+1)

    # z-runs: maximal k-segments where both r0,r1 step by a constant d in -1..1
    runs = []
    k = 0
    while k < N:
        step = 0
        if k + 1 < N:
            d = int(r0[k + 1] - r0[k])
            if d == int(r1[k + 1] - r1[k]) and abs(d) <= 1:
                step = d
        ln = 1
        while (k + ln < N
               and int(r0[k + ln] - r0[k]) == step * ln
               and int(r1[k + ln] - r1[k]) == step * ln):
            ln += 1
        runs.append((k, ln, int(r0[k]), int(r1[k]), step))
        k += ln

    return dict(idx=idx, w00=w00, w01=w01, w10=w10, w11=w11, fw=fw, runs=runs)


def _pack_idxs(idx_flat):
    """int16 dma_gather index layout: element i at [i%16, i//16], replicated to 128 partitions."""
    t = idx_flat.reshape(-1, 16).T.astype(np.int16)  # [16, n/16]
    return np.ascontiguousarray(np.tile(t, (8, 1)))  # [128, n/16]


def _pack_wsm(tables):
    """[16384, 512] fp16 mix weights, row n (= i*128+j) =
    [w00 x128, w10 x128, w01 x128, w11 x128] (order matches the gathered
    element's (q,p)-corner slices at strides of 256). Materialized over r so
    the DVE mix multiply runs in fp16 2x mode (every operand needs 2-byte
    dtype with a stride-1 innermost dim; the v-broadcast uses a stride-0
    MIDDLE dim, which 2x mode permits)."""
    w = np.stack([tables[nm].reshape(-1) for nm in ("w00", "w10", "w01", "w11")],
                 axis=1).astype(np.float16)          # [n, 4]
    return np.ascontiguousarray(
        np.broadcast_to(w[:, :, None], (NROWS, 4, 128)).reshape(NROWS, 512))


def _build_volquad(v0, v1):
    """v0,v1: [128,128,128] f32. Returns [16512, 512] fp16 with row (p,q) =
    [v0[p,q,:], v1[p,q,:], v0[p1,q,:], v1[p1,q,:]], p1 = min(p+1, 127)."""
    p1 = np.minimum(np.arange(N) + 1, N - 1)
    vq = np.empty((N, N, 4, N), dtype=np.float16)
    vq[:, :, 0] = v0
    vq[:, :, 1] = v1
    vq[:, :, 2] = v0[p1]
    vq[:, :, 3] = v1[p1]
    out = np.zeros((NROWSQ, 4 * N), dtype=np.float16)
    out[:NROWS] = vq.reshape(NROWS, 4 * N)
    return out


def _build_program(tables, reps=1):
    """Raw-Bass program with explicit semaphores.

    Engine streams:
      sync   (SP HWDGE): const loads (idx, weights, fw tables)
      gpsimd (SWDGE):    one dma_gather per call (1024 x 2KiB descriptors)
      vector (DVE):      4-corner mix + z-staircase
      scalar (ACT HWDGE): fp16 output writes
    """
    import concourse.bass as bass
    from concourse import bacc, mybir

    runs = tables["runs"]
    f16 = mybir.dt.float16
    i16 = mybir.dt.int16
    AP = bass.AP

    nc = bacc.Bacc("TRN2", target_bir_lowering=False, debug=False, num_devices=8)

    volquad = nc.dram_tensor("volquad", [NROWSQ, 4 * N], f16, kind="ExternalInput")
    idx_dram = nc.dram_tensor("idx", [128, NROWS // 16], i16, kind="ExternalInput")
    wb_dram = nc.dram_tensor("wsm", [NROWS, 512], f16, kind="ExternalInput")
    fw_dram = nc.dram_tensor("fwrep", [2, 128, 128], f16, kind="ExternalInput")
    out_dram = [nc.dram_tensor(f"out{v}", [NROWS, N], f16, kind="ExternalOutput")
                for v in range(2)]

    idx_t = nc.alloc_sbuf_tensor("idx_t", [128, NROWS // 16], i16)
    fw_t = [nc.alloc_sbuf_tensor(f"fw{c}", [128, 128], f16) for c in range(2)]
    NB = 3
    G = [nc.alloc_sbuf_tensor(f"G{p}", [128, SLOTS * ELEM], f16) for p in range(NB)]
    WB = [nc.alloc_sbuf_tensor(f"WB{p}", [128, SLOTS * 512], f16) for p in range(NB)]
    M = nc.alloc_sbuf_tensor("M", [128, SLOTS * ELEM], f16)
    A1 = nc.alloc_sbuf_tensor("A1", [128, SLOTS * 512], f16)
    R0 = nc.alloc_sbuf_tensor("R0", [128, SLOTS * 256], f16)
    t1 = nc.alloc_sbuf_tensor("t1", [128, SLOTS * 256], f16)
    acc = [nc.alloc_sbuf_tensor(f"acc{p}", [128, SLOTS * 256], f16) for p in range(4)]

    TOT = NCALL * reps
    nrows_ap = NROWSQ - 2 + 1

    from contextlib import ExitStack
    with ExitStack() as _sctx:
        block = _sctx.enter_context(nc.Block())
        s_wf = _sctx.enter_context(nc.semaphore("s_wf"))
        s_g = [_sctx.enter_context(nc.semaphore(f"s_g{p}")) for p in range(NB)]
        s_wb = [_sctx.enter_context(nc.semaphore(f"s_wb{p}")) for p in range(NB)]
        s_v = _sctx.enter_context(nc.semaphore("s_v"))
        s_mix = _sctx.enter_context(nc.semaphore("s_mix"))
        s_o = [_sctx.enter_context(nc.semaphore(f"s_o{p}")) for p in range(4)]

        @block.sync
        def _(sync):
            sync.dma_start(idx_t.ap(), idx_dram.ap()).then_inc(s_wf, 16)
            for c in range(2):
                sync.dma_start(fw_t[c].ap(),
                               AP(fw_dram, c * 128 * 128, [[128, 128], [1, 128]])
                               ).then_inc(s_wf, 16)
            for C in range(TOT):
                cl = C % NCALL
                if C >= NB:
                    sync.wait_ge(s_mix, C - NB + 1)
                sync.dma_start(
                    AP(WB[C % NB], 0, [[SLOTS * 512, 128], [512, SLOTS], [1, 512]]),
                    AP(wb_dram, cl * NIDX * 512,
                       [[512, 128], [128 * 512, SLOTS], [1, 512]]),
                ).then_inc(s_wb[C % NB], 16)
            for p in range(4):
                sync.wait_ge(s_o[p], 32 * (TOT // 4))

        @block.gpsimd
        def _(gpsimd):
            nreg = gpsimd.to_reg(NIDX)
            gpsimd.wait_ge(s_wf, 48)
            sv = AP(volquad, 0, [[ESTEP, nrows_ap], [1, ELEM]])
            for C in range(TOT):
                cl = C % NCALL
                if C >= NB:
                    gpsimd.wait_ge(s_mix, C - NB + 1)
                gpsimd.dma_gather(
                    AP(G[C % NB], 0, [[SLOTS * ELEM, 128], [ELEM, SLOTS], [1, ELEM]]),
                    sv,
                    AP(idx_t, cl * (NIDX // 16), [[NROWS // 16, 128], [1, NIDX // 16]]),
                    NIDX, nreg, ELEM, elem_step=ESTEP,
                ).then_inc(s_g[C % NB], 16)

        @block.vector
        def _(vector):
            mult = mybir.AluOpType.mult
            VC = [0]

            def vsync(last_ins):
                # DVE pipeline does not interlock same-engine RAW hazards;
                # self-semaphore between dependent phases.
                last_ins.then_inc(s_v, 1)
                VC[0] += 1
                vector.wait_ge(s_v, VC[0])

            vector.wait_ge(s_wf, 48)
            mdims = [[SLOTS * ELEM, 128], [256, 4 * SLOTS], [128, 2], [1, 128]]
            wdims = [[SLOTS * 512, 128], [128, 4 * SLOTS], [0, 2], [1, 128]]
            for C in range(TOT):
                vector.wait_ge(s_g[C % NB], 16 * (C // NB + 1))
                vector.wait_ge(s_wb[C % NB], 16 * (C // NB + 1))
                if C >= 1:
                    # M/A1/R0/t1 WAR vs previous call (same-engine pipeline)
                    vector.wait_ge(s_mix, C)
                if C >= 4:
                    vector.wait_ge(s_o[C % 4], 32 * (C // 4))
                A = acc[C % 4]
                # mix: one packed fp16 multiply (2x mode), then pair-sum tree
                vsync(vector.tensor_tensor(AP(M, 0, mdims), AP(G[C % NB], 0, mdims),
                                           AP(WB[C % NB], 0, wdims), mult))
                vsync(vector.tensor_add(
                    AP(A1, 0, [[SLOTS * 512, 128], [512, SLOTS], [1, 512]]),
                    AP(M, 0, [[SLOTS * ELEM, 128], [ELEM, SLOTS], [1, 512]]),
                    AP(M, 512, [[SLOTS * ELEM, 128], [ELEM, SLOTS], [1, 512]])))
                vsync(vector.tensor_add(
                    AP(R0, 0, [[SLOTS * 256, 128], [256, SLOTS], [1, 256]]),
                    AP(A1, 0, [[SLOTS * 512, 128], [512, SLOTS], [1, 256]]),
                    AP(A1, 256, [[SLOTS * 512, 128], [512, SLOTS], [1, 256]])))
                # z-staircase: view R0/acc as [128, 16 (slot*v), 128 (r|k)]
                last = None
                for (ks, ln, r0s, r1s, st) in runs:
                    adst = AP(A, ks, [[SLOTS * 256, 128], [128, 2 * SLOTS], [1, ln]])
                    tdst = AP(t1, ks, [[SLOTS * 256, 128], [128, 2 * SLOTS], [1, ln]])
                    v0 = AP(R0, r0s, [[SLOTS * 256, 128], [128, 2 * SLOTS], [st, ln]])
                    v1 = AP(R0, r1s, [[SLOTS * 256, 128], [128, 2 * SLOTS], [st, ln]])
                    f0 = AP(fw_t[0], ks, [[128, 128], [0, 2 * SLOTS], [1, ln]])
                    f1 = AP(fw_t[1], ks, [[128, 128], [0, 2 * SLOTS], [1, ln]])
                    vector.tensor_tensor(adst, v0, f0, mult)
                    last = vector.tensor_tensor(tdst, v1, f1, mult)
                vsync(last)
                for (ks, ln, r0s, r1s, st) in runs:
                    adst = AP(A, ks, [[SLOTS * 256, 128], [128, 2 * SLOTS], [1, ln]])
                    tsrc = AP(t1, ks, [[SLOTS * 256, 128], [128, 2 * SLOTS], [1, ln]])
                    last = vector.tensor_add(adst, adst, tsrc)
                last.then_inc(s_mix, 1)

        @block.scalar
        def _(scalar):
            for C in range(TOT):
                cl = C % NCALL
                scalar.wait_ge(s_mix, C + 1)
                for v in range(2):
                    scalar.dma_start(
                        AP(out_dram[v], cl * NIDX * N, [[N, 128], [128 * N, SLOTS], [1, N]]),
                        AP(acc[C % 4], v * 128, [[SLOTS * 256, 128], [256, SLOTS], [1, N]]),
                    ).then_inc(s_o[C % 4], 16)

    nc.compile()
    return nc


def _exact_label_fixup(label_g, theta, lab_f, out_bool):
    """Recompute voxels of |lab_f - 0.5| < BAND in the reference's exact f32
    arithmetic order."""
    cand = np.abs(lab_f - np.float32(0.5)) < BAND
    if not cand.any():
        return out_bool
    bb, ii, jj, kk = np.nonzero(cand.reshape(-1, N, N, N))
    v = _exact_reference_values(label_g, theta, bb, ii, jj, kk)
    out_bool.reshape(-1, N, N, N)[bb, ii, jj, kk] = v > np.float32(0.5)
    return out_bool


def _exact_reference_values(vol_g, theta, bb, ii, jj, kk):
    """Reference-order f32 trilinear values at selected voxels.

    Replicates: grid einsum (x*t0 + y*t1 + z*t2, left-assoc f32) + t3; unnorm;
    8-corner accumulation in (z,y,x) order with w=(wz*wy)*wx, out += v*w.
    """
    f32 = np.float32
    t = np.arange(N, dtype=f32)
    xn = ((f32(2.0) * t + f32(1.0)) / f32(N) - f32(1.0)).astype(f32)
    th = theta.astype(f32)

    x = xn[ii]; y = xn[jj]; z = xn[kk]

    def fma32(a, b, c):
        return (np.float64(a) * np.float64(b) + c.astype(np.float64)).astype(f32)

    def comp(r):
        a = fma32(y, th[r, 1], (x * th[r, 0]).astype(f32))
        a = fma32(z, th[r, 2], a)
        return (a + th[r, 3]).astype(f32)
    gx, gy, gz = comp(0), comp(1), comp(2)

    def unnorm(c):
        return np.clip(((c + f32(1.0)) * f32(N) - f32(1.0)) * f32(0.5), f32(0.0), f32(N - 1))
    ux, uy, uz = unnorm(gx), unnorm(gy), unnorm(gz)
    x0 = np.floor(ux); y0 = np.floor(uy); z0 = np.floor(uz)
    fx = (ux - x0).astype(f32); fy = (uy - y0).astype(f32); fz = (uz - z0).astype(f32)
    x0i = x0.astype(np.int64); y0i = y0.astype(np.int64); z0i = z0.astype(np.int64)
    x1i = np.minimum(x0i + 1, N - 1); y1i = np.minimum(y0i + 1, N - 1); z1i = np.minimum(z0i + 1, N - 1)

    vol = vol_g.reshape(-1, N, N, N)
    out = np.zeros(bb.shape, f32)
    one = f32(1.0)
    for zi, wz in ((z0i, (one - fz).astype(f32)), (z1i, fz)):
        for yi, wy in ((y0i, (one - fy).astype(f32)), (y1i, fy)):
            for xi, wx in ((x0i, (one - fx).astype(f32)), (x1i, fx)):
                vals = vol[bb, xi, yi, zi]
                w = ((wz * wy).astype(f32) * wx).astype(f32)
                out = (out + (vals * w).astype(f32)).astype(f32)
    return out


def _host_fallback(input_g, label_g, transform):
    """Arbitrary-transform fallback: full reference computation on host."""
    bb, ii, jj, kk = np.meshgrid(np.arange(8), np.arange(N), np.arange(N), np.arange(N), indexing="ij")
    bb, ii, jj, kk = (a.reshape(-1) for a in (bb, ii, jj, kk))
    theta = transform[:3].astype(np.float32)
    aug_inp = _exact_reference_values(input_g, theta, bb, ii, jj, kk).reshape(8, 1, N, N, N)
    lab = _exact_reference_values(label_g, theta, bb, ii, jj, kk).reshape(8, 1, N, N, N)
    return aug_inp.astype(np.float32), lab > np.float32(0.5)


def _make_in_maps(input_g, label_g, tables):
    common = {
        "idx": _pack_idxs(tables["idx"].reshape(-1)),
        "wsm": _pack_wsm(tables),
        "fwrep": np.stack([np.tile(1.0 - tables["fw"], (128, 1)),
                           np.tile(tables["fw"], (128, 1))]).astype(np.float16),
    }
    return [dict(common, volquad=_build_volquad(input_g[b, 0], label_g[b, 0]))
            for b in range(8)]


def kernel(input_g, label_g, transform):
    input_g = np.ascontiguousarray(input_g, dtype=np.float32)
    label_g = np.ascontiguousarray(label_g, dtype=np.float32)
    transform = np.asarray(transform, dtype=np.float32)
    theta = transform[:3]

    structured = (abs(float(theta[0, 2])) < 1e-12 and abs(float(theta[1, 2])) < 1e-12
                  and abs(float(theta[2, 0])) < 1e-12 and abs(float(theta[2, 1])) < 1e-12)
    if not structured:
        return _host_fallback(input_g, label_g, transform)

    from concourse.bass_utils import run_bass_kernel_spmd

    tables = _host_tables(theta)
    key = transform.tobytes()
    if key not in _CACHE:
        _CACHE[key] = _build_program(tables)
    nc = _CACHE[key]

    in_maps = _make_in_maps(input_g, label_g, tables)
    res = run_bass_kernel_spmd(nc, in_maps, core_ids=list(range(8)))

    aug_inp = np.empty((8, 1, N, N, N), np.float32)
    lab_f = np.empty((8, 1, N, N, N), np.float32)
    for b in range(8):
        aug_inp[b, 0] = res.results[b]["out0"].astype(np.float32).reshape(N, N, N)
        lab_f[b, 0] = res.results[b]["out1"].astype(np.float32).reshape(N, N, N)

    out_bool = lab_f > np.float32(0.5)
    out_bool = _exact_label_fixup(label_g, theta, lab_f, out_bool)
    return aug_inp, out_bool
